# revision 1
# baseline (speedup 1.0000x reference)
"""Trainium2 Bass kernel for nn_Conv2d (B=32, 256->256, 56x56, 3x3, pad=1) + bias.

Strategy
--------
Data-parallel over batch: 4 images per NeuronCore x 8 cores; weights/bias
replicated; no collectives.

Per core, the conv is computed as shifted matmuls: the input is zero-padded on
the HOST to 58-wide rows (59 rows x 58 cols per image-channel, flattened to
3422), so output position (h, w) <-> flat index h*58+w, and the 3x3 tap
(kh, kw) contribution is a matmul against the padded input shifted by the
constant offset kh*58+kw.  Each output tile [128 couts x 464 positions]
accumulates 2 (cin chunks) x 9 (taps) = 18 matmuls in one PSUM bank
(3248 = 7*464 padded output positions per image; columns w in {56,57} are
junk and stripped on the host).  Matmuls run as float32r (1 cycle/row on the
PE at N>=256; ~1.4e-4 relative error, measured on HW).  Bias is fused into
the PSUM->SBUF eviction via ScalarE activation(Identity, bias=...).
"""

import numpy as np

import concourse.bacc as bacc
import concourse.tile as tile
import concourse.mybir as mybir
from concourse.bass_utils import run_bass_kernel_spmd

F32 = mybir.dt.float32
F32R = mybir.dt.float32r

B, CIN, COUT, H, W, K = 32, 256, 256, 56, 56, 3
NCORES = 8
BPC = B // NCORES          # images per core
WP = W + 2                 # padded row width (58)
HP = H + 3                 # padded rows (59): 1 top, 2 bottom (tail tap reads)
XF = HP * WP               # padded flat length per image-channel (3422)
OF = H * WP                # padded output flat length (3248)
NT = 7                     # output tiles per (img, cout-chunk)
NFREE = OF // NT           # 464 positions per matmul (>=256 keeps f32r fast)

_CACHE = {}


def _build():
    if "nc" in _CACHE:
        return _CACHE["nc"]
    nc = bacc.Bacc("TRN2", target_bir_lowering=False, debug=False,
                   num_swdge_queues=4)
    x_d = nc.dram_tensor("x", [BPC, CIN, XF], F32R, kind="ExternalInput").ap()
    w_d = nc.dram_tensor("w", [K * K, CIN, COUT], F32R, kind="ExternalInput").ap()
    b_d = nc.dram_tensor("b", [COUT], F32, kind="ExternalInput").ap()
    o_d = nc.dram_tensor("o", [BPC, COUT, OF], F32, kind="ExternalOutput").ap()

    XLOAD = 3366  # matmuls never read past 3365

    with tile.TileContext(nc) as tc:
        with (
            tc.tile_pool(name="wp", bufs=1) as wp,
            tc.tile_pool(name="xp", bufs=6) as xp,
            tc.tile_pool(name="op", bufs=2) as op,
            tc.tile_pool(name="pp", bufs=8, space="PSUM") as pp,
        ):
            # DMA trigger instructions cost ~0.6us EACH on the issuing
            # engine, so issue in parallel from both HWDGE engines:
            # sync carries ci=0 traffic, scalar carries ci=1.
            eng = [nc.sync, nc.scalar]

            bias_t = wp.tile([128, 2], F32)
            # weights [cin-in-chunk, cin_chunk, tap, cout] in per-(cc,ci,tap)
            # 64KB DMAs: the cc=0 half (1.18MB) is all the first compute wave
            # needs; cc=1 arrives during it.
            w_t = wp.tile([128, 2, K * K, COUT], F32R)

            def w_dma(e, ci, t, cc):
                e.dma_start(
                    out=w_t[:, ci, t, cc * 128:(cc + 1) * 128],
                    in_=w_d[t, ci * 128:(ci + 1) * 128, cc * 128:(cc + 1) * 128],
                )

            def x_dma(e, xs, img, ci, lo, hi):
                e.dma_start(
                    out=xs[ci][:, lo:hi],
                    in_=x_d[img, ci * 128:(ci + 1) * 128, lo:hi],
                )

            def alloc_x():
                xs = []
                for ci in range(2):
                    x_t = xp.tile([128, XF], F32R, tag="x")
                    xs.append(x_t)
                return xs

            # slice boundaries: nt-pair p depends only on x up to
            # 582+464*(2p+1), so early pairs unblock as slices land
            xsl = [0, 291, 582, 1046, 1510, 1974, 2438, 2902, XLOAD]

            def load_img(img):
                xs = alloc_x()
                for s in range(len(xsl) - 1):
                    for ci in range(2):
                        x_dma(eng[ci], xs, img, ci, xsl[s], xsl[s + 1])
                return xs

            def load_img0():
                # Hand-scheduled startup: DMA triggers cost ~0.65us each on
                # the issuing engine; sync (ci=0) and scalar (ci=1) carry
                # first-wave weights + x interleaved by consumption time
                xs = alloc_x()
                for ci in range(2):
                    e = eng[ci]
                    w_dma(e, ci, 0, 0)
                    x_dma(e, xs, 0, ci, xsl[0], xsl[1])
                    x_dma(e, xs, 0, ci, xsl[1], xsl[2])
                    x_dma(e, xs, 0, ci, xsl[2], xsl[3])
                    w_dma(e, ci, 1, 0)
                    w_dma(e, ci, 2, 0)
                    x_dma(e, xs, 0, ci, xsl[3], xsl[4])
                    w_dma(e, ci, 3, 0)
                    w_dma(e, ci, 4, 0)
                    x_dma(e, xs, 0, ci, xsl[4], xsl[5])
                    w_dma(e, ci, 5, 0)
                    w_dma(e, ci, 6, 0)
                    x_dma(e, xs, 0, ci, xsl[5], xsl[6])
                    w_dma(e, ci, 7, 0)
                    w_dma(e, ci, 8, 0)
                    e.dma_start(out=bias_t[:, ci:ci + 1],
                                in_=b_d[ci * 128:(ci + 1) * 128])
                    x_dma(e, xs, 0, ci, xsl[6], xsl[7])
                    x_dma(e, xs, 0, ci, xsl[7], xsl[8])
                return xs

            def do_group(xs, cc, o_t, img, nts, fine_stores=False):
                """One PSUM accumulation wave over nt tiles `nts` (1 or 2),
                sharing each weight tile across the wave to halve LDWEIGHTS
                pressure on the PE."""
                pss = []
                for nt in nts:
                    ps = pp.tile([128, NFREE], F32, tag="ps")
                    pss.append(ps)
                for mi, (ci, t) in enumerate(
                    [(ci, t) for ci in range(2) for t in range(K * K)]
                ):
                    kh, kw = divmod(t, K)
                    for ps, nt in zip(pss, nts):
                        off = nt * NFREE + kh * WP + kw
                        nc.tensor.matmul(
                            ps,
                            w_t[:, ci, t, cc * 128:(cc + 1) * 128],
                            xs[ci][:, off:off + NFREE],
                            start=(mi == 0),
                            stop=(mi == 17),
                        )
                for ps, nt in zip(pss, nts):
                    # bias-add + PSUM eviction on the otherwise-idle DVE
                    nc.vector.tensor_scalar_add(
                        o_t[:, nt * NFREE:(nt + 1) * NFREE],
                        ps,
                        bias_t[:, cc:cc + 1],
                    )
                    # store each nt slice as soon as its bias-add finishes,
                    # halves split across the issue engines (quarters for
                    # the final group so the drain tail stays short)
                    nsp = 4 if fine_stores else 2
                    q = NFREE // nsp
                    for s in range(nsp):
                        h0 = nt * NFREE + s * q
                        eng[s % 2].dma_start(
                            out=o_d[img, cc * 128:(cc + 1) * 128, h0:h0 + q],
                            in_=o_t[:, h0:h0 + q],
                        )

            for img in range(BPC):
                if img == 0:
                    xs = load_img0()
                    # cc=1 weights via SWDGE, needed ~27us in
                    for ci in range(2):
                        for t in range(K * K):
                            w_dma(nc.gpsimd, ci, t, 1)
                else:
                    xs = load_img(img)
                for cc in range(2):
                    o_t = op.tile([128, OF], F32, tag="o")
                    last = img == BPC - 1 and cc == 1
                    for nts in ([0, 1], [2, 3], [4, 5], [6]):
                        do_group(xs, cc, o_t, img, nts,
                                 fine_stores=last and nts == [6])
    nc.compile()
    _CACHE["nc"] = nc
    return nc


def make_in_maps(inp, kernel, bias):
    xpad = np.zeros((B, CIN, HP, WP), np.float32)
    xpad[:, :, 1:1 + H, 1:1 + W] = inp
    xflat = xpad.reshape(B, CIN, XF)
    # [cout, cin, kh, kw] -> [tap(kh*3+kw), cin, cout]
    w_dev = np.ascontiguousarray(
        np.asarray(kernel, np.float32).transpose(2, 3, 1, 0).reshape(K * K, CIN, COUT)
    )
    b_dev = np.ascontiguousarray(np.asarray(bias, np.float32))
    return [
        {"x": np.ascontiguousarray(xflat[c * BPC:(c + 1) * BPC]),
         "w": w_dev, "b": b_dev}
        for c in range(NCORES)
    ]


def assemble(results):
    o = np.concatenate([results[c]["o"] for c in range(NCORES)], axis=0)
    return np.ascontiguousarray(
        o.reshape(B, COUT, H, WP)[:, :, :, :W].astype(np.float32)
    )


def kernel(inp, kernel, bias):
    nc = _build()
    in_maps = make_in_maps(inp, kernel, bias)
    r = run_bass_kernel_spmd(nc, in_maps, core_ids=list(range(NCORES)))
    return assemble(r.results)

